# revision 37
# baseline (speedup 1.0000x reference)
"""Trainium2 Bass kernel for a dense transformer block (pre-LN, causal attn).

Sharding across 8 NeuronCores:
  - sequence-sharded: LN1, QKV projection, attn-output proj, LN2, MLP
    (core c owns rows [512c, 512c+512) of T=4096)
  - head-sharded: attention itself (core c owns heads 2c, 2c+1 over all T)
  - collectives: a zero-cost barrier collective at t=0 absorbs the
    per-launch rank-arrival skew; A2A#1 is split into a qk collective
    (fired early) and a v collective that flies while attention S starts;
    A2A#2 carries sender-normalized y in transposed orientation. All
    three data collectives move fp8 payloads.

fp8 (e4m3) with DoubleRow (2 contraction-tiles per matmul, 2x stream) is
used for the QKV projection, PV, and the attention-output projection;
S runs in plain fp8; the MLP stays bf16 (fp8 there would breach the
error budget - measured on the reference inputs). Scales: h*16, W*128,
q/k/v*32, exp*16 (via +ln16 in the exp bias), y*32.

LayerNorm scale/bias are folded into the following matmul weights and
biases on the host; the device LN is bn_stats/bn_aggr + one activation.
Causal masking is applied additively (-1e9) on the S PSUM before exp.
Softmax skips max-subtraction (scores bounded by construction).
"""

import sys

for _p in ("/opt/trn_rl_repo", "/root/.axon_site/_ro/trn_rl_repo"):
    if _p not in sys.path:
        sys.path.insert(0, _p)

import numpy as np
import ml_dtypes

import concourse.bass as bass
import concourse.mybir as mybir
import concourse.tile as tile
from concourse import bacc, bass_utils
from concourse.bass import ds, ts

F32 = mybir.dt.float32
BF16 = mybir.dt.bfloat16
FP8 = mybir.dt.float8e4
DR = mybir.MatmulPerfMode.DoubleRow
AF = mybir.ActivationFunctionType
ALU = mybir.AluOpType

# model dims
D = 1024
T = 4096
H = 16
DH = 64
DFF = 4096
EPS = 1e-5
NCORES = 8
CHUNK = T // NCORES        # 512 rows per core
RG = CHUNK // 128          # 4 row groups
NQT = T // 128             # 32 key subblocks (for the 2 owned heads)
DT = D // 128              # 8 d-tiles
NFFT = DFF // 128          # 32 dff tiles

S_H = 16.0                 # LN1 output scale
S_W = 128.0                # fp8 weight scale
S_QKV = 32.0               # q/k/v scale
S_E = 16.0                 # exp scale (via +ln(S_E) in exp bias)
S_Y = 32.0                 # normalized-y scale

_cached = {}


def _build_nc():
    nc = bacc.Bacc("TRN2", target_bir_lowering=False)

    x_c = nc.dram_tensor("x_c", [CHUNK, D], F32, kind="ExternalInput")
    w_qk = nc.dram_tensor("w_qk", [D, 2 * D], FP8, kind="ExternalInput")
    w_v = nc.dram_tensor("w_v", [128, 2, DT, 512], FP8, kind="ExternalInput")
    w_proj = nc.dram_tensor("w_proj", [128, 2, DT, 512], FP8, kind="ExternalInput")
    w_fc = nc.dram_tensor("w_fc", [D, DFF], FP8, kind="ExternalInput")
    w_fc2 = nc.dram_tensor("w_fc2", [DFF, D], BF16, kind="ExternalInput")
    b_qk = nc.dram_tensor("b_qk", [128, 16], F32, kind="ExternalInput")
    b_fc = nc.dram_tensor("b_fc", [128, NFFT], F32, kind="ExternalInput")
    bv_bc = nc.dram_tensor("bv_bc", [128, D], F32, kind="ExternalInput")
    bproj_bc = nc.dram_tensor("bproj_bc", [128, D], F32, kind="ExternalInput")
    bfc2_bc = nc.dram_tensor("bfc2_bc", [128, D], F32, kind="ExternalInput")
    ntri_in = nc.dram_tensor("ntri", [128, 128], F32, kind="ExternalInput")
    ident_in = nc.dram_tensor("ident", [128, 128], BF16, kind="ExternalInput")

    out_c = nc.dram_tensor("out_c", [CHUNK, D], F32, kind="ExternalOutput")

    x_v = x_c.rearrange("(rg p) d -> p rg d", p=128)
    out_v = out_c.rearrange("(rg p) d -> p rg d", p=128)
    wqk_v = w_qk.rearrange("(dt p) c -> p dt c", p=128)
    wfc_v = w_fc.rearrange("(dt p) c -> p dt c", p=128)
    wfc2_v = w_fc2.rearrange("(ft p) c -> p ft c", p=128)

    with tile.TileContext(nc) as tc:
        with (
            tc.tile_pool(name="const", bufs=1) as const,
            tc.tile_pool(name="persist", bufs=1) as persist,
            tc.tile_pool(name="dram", bufs=1, space="DRAM") as dram,
        ):
            # barrier: absorb the per-launch rank skew while the PE does
            # LN1/QKV, so the first real collective doesn't pay it
            bar_in = dram.tile([NCORES, 128], BF16)
            bar_out = dram.tile([NCORES, 128], BF16)
            nc.gpsimd.collective_compute(
                "AllToAll",
                ALU.bypass,
                ins=[bar_in.opt()],
                outs=[bar_out.opt()],
                replica_groups=[list(range(NCORES))],
            )

            x_sb = persist.tile([128, RG, D], F32, tag="x_sb")
            ident = const.tile([128, 128], BF16)
            nc.sync.dma_start(x_sb[:, 0], x_v[:, 0])
            nc.sync.dma_start(ident[:], ident_in[:])
            for rg in range(1, RG):
                nc.sync.dma_start(x_sb[:, rg], x_v[:, rg])
            ntri = const.tile([128, 128], F32)
            nc.sync.dma_start(ntri[:], ntri_in[:])
            bv = const.tile([128, D], F32)
            bqk_sb = const.tile([128, 16], F32)
            bfc_sb = const.tile([128, NFFT], F32)
            bproj = const.tile([128, D], F32)
            bfc2 = const.tile([128, D], F32)
            eps_sb = const.tile([128, 1], F32)
            ln16_sb = const.tile([128, 1], F32)
            nc.vector.memset(eps_sb[:], EPS)
            nc.vector.memset(ln16_sb[:], float(np.log(S_E)))
            nc.sync.dma_start(bv[:], bv_bc[:])
            nc.sync.dma_start(bqk_sb[:], b_qk[:])

            def layernorm(pool, pt_pool, src_col, hT, rg, out_scale, dtype):
                """one row-group of LN: src [128, D] f32 -> hT cols,
                normalized only (affine folded into weights), scaled by
                out_scale, cast to dtype at the PSUM->SBUF copy."""
                xin = src_col[:, rg]
                st6 = pool.tile([128, 2, 6], F32, tag="ln_st6")
                for g in range(2):
                    nc.vector.bn_stats(st6[:, g], xin[:, ds(g * 512, 512)])
                mv = pool.tile([128, 2], F32, tag="ln_mv")
                nc.vector.bn_aggr(mv[:], st6[:])
                std = pool.tile([128, 1], F32, tag="ln_std")
                nc.scalar.activation(std[:], mv[:, 1:2], AF.Sqrt, bias=eps_sb[:])
                rstd = pool.tile([128, 1], F32, tag="ln_rstd")
                nc.vector.reciprocal(rstd[:], std[:])
                nmr = pool.tile([128, 1], F32, tag="ln_nmr")
                nc.vector.scalar_tensor_tensor(
                    nmr[:], mv[:, 0:1], -1.0, rstd[:], ALU.mult, ALU.mult
                )
                h = pool.tile([128, D], BF16, tag="ln_h")
                nc.scalar.activation(
                    h[:], xin, AF.Identity, bias=nmr[:], scale=rstd[:]
                )
                pt = pt_pool.tile([128, 1024], BF16, tag="pt")
                for d in range(DT):
                    nc.tensor.transpose(pt[:, ts(d, 128)], h[:, ts(d, 128)], ident[:])
                nc.scalar.activation(
                    hT[:, :, ds(rg * 128, 128)],
                    pt[:].rearrange("p (a b) -> p a b", a=DT),
                    AF.Copy, scale=out_scale,
                )

            # ================= Phase A/B: LN1, QK -> A2A#1qk, V -> A2A#1v ====
            a2a1q_in = dram.tile([NCORES, 2, 128, 512], FP8)
            a2a1q_out = dram.tile([NCORES, 2, 128, 512], FP8)
            a2a1v_in = dram.tile([NCORES, 128, 512], FP8)
            a2a1v_out = dram.tile([NCORES, 128, 512], FP8)
            with (
                tc.tile_pool(name="ph_a", bufs=2) as ph_a,
                tc.tile_pool(name="ps1", bufs=3, space="PSUM") as ps1,
                tc.tile_pool(name="pspt1", bufs=2, space="PSUM") as pspt1,
            ):
                hT = ph_a.tile([128, DT, CHUNK], FP8, tag="hT", bufs=1)
                wqk_sb = ph_a.tile([128, DT, 2 * D], FP8, tag="wqk_sb", bufs=1)
                nc.sync.dma_start(wqk_sb[:], wqk_v[:])
                for rg in range(RG):
                    layernorm(ph_a, pspt1, x_sb, hT, rg, S_H, FP8)

                # q/k: DoubleRow over d-tile pairs; out fp8 * S_QKV.
                # staging DMAs issue from the Activation engine so the SP
                # DMA queue never blocks the PE's weight feed.
                qkT = ph_a.tile([128, 16, CHUNK], FP8, tag="qkT", bufs=1)
                for o in range(16):
                    ps = ps1.tile([128, 512], F32, tag="mm")
                    for dp in range(DT // 2):
                        nc.tensor.matmul(
                            ps[:],
                            wqk_sb[:, ds(2 * dp, 2), ds(o * 128, 128)],
                            hT[:, ds(2 * dp, 2)],
                            start=(dp == 0), stop=(dp == DT // 2 - 1),
                            perf_mode=DR,
                        )
                    nc.scalar.activation(
                        qkT[:, o], ps[:], AF.Identity,
                        bias=bqk_sb[:, o : o + 1], scale=S_QKV / (S_H * S_W),
                    )
                    nc.scalar.dma_start(a2a1q_in[o % 8, o // 8], qkT[:, o])
                nc.gpsimd.collective_compute(
                    "AllToAll",
                    ALU.bypass,
                    ins=[a2a1q_in.opt()],
                    outs=[a2a1q_out.opt()],
                    replica_groups=[list(range(NCORES))],
                )
                # late-needed consts: keep the DMA queue clear early on
                nc.sync.dma_start(bfc_sb[:], b_fc[:])
                nc.sync.dma_start(bproj[:], bproj_bc[:])
                nc.sync.dma_start(bfc2[:], bfc2_bc[:])

                # v while the qk collective flies
                v_nat = ph_a.tile([128, RG, D], FP8, tag="v_nat", bufs=1)
                wv_t = ph_a.tile([128, 2, DT, 512], FP8, tag="w_vt", bufs=1)
                nc.sync.dma_start(
                    wv_t[:].rearrange("p a b w -> p (a b) w"),
                    w_v.rearrange("p a b w -> p (a b) w"),
                )
                for rg in range(RG):
                    for vh in range(2):
                        ps = ps1.tile([128, 512], F32, tag="mm")
                        for dp in range(DT // 2):
                            nc.tensor.matmul(
                                ps[:],
                                hT[:, ds(2 * dp, 2), ds(rg * 128, 128)],
                                wv_t[:, vh, ds(2 * dp, 2)],
                                start=(dp == 0), stop=(dp == DT // 2 - 1),
                                perf_mode=DR,
                            )
                        nc.vector.scalar_tensor_tensor(
                            v_nat[:, rg, ds(vh * 512, 512)], ps[:],
                            S_QKV / (S_H * S_W), bv[:, ds(vh * 512, 512)],
                            ALU.mult, ALU.add,
                        )
                for j in range(NCORES):
                    nc.sync.dma_start(
                        a2a1v_in[j].rearrange("p (rg w) -> p rg w", rg=RG),
                        v_nat[:, :, ds(j * 128, 128)],
                    )
                nc.gpsimd.collective_compute(
                    "AllToAll",
                    ALU.bypass,
                    ins=[a2a1v_in.opt()],
                    outs=[a2a1v_out.opt()],
                    replica_groups=[list(range(NCORES))],
                )

            # ============ Phase D: attention (2 owned heads, all T) ==========
            # Per dest chunk qg (descending, big first), per head, key-tile
            # PAIRS: two S matmuls (plain fp8) -> additive causal mask on
            # PSUM -> one exp (out fp8 * S_E) -> one DoubleRow PV matmul.
            # Sender-side normalize (reciprocal + partition broadcast).
            a2a2_in = dram.tile([NCORES, 2, 64, 512], FP8)
            a2a2_out = dram.tile([NCORES, 2, 64, 512], FP8)
            wp_sb = persist.tile([128, 2, DT, 512], FP8, tag="wp_sb")
            wfc_sb = persist.tile([128, DT, DFF], FP8, tag="wfc_sb")
            with (
                tc.tile_pool(name="ph_d", bufs=2) as ph_d,
                tc.tile_pool(name="ps_s", bufs=2, space="PSUM") as ps_s,
                tc.tile_pool(name="ps_y", bufs=2, space="PSUM") as ps_y,
            ):
                qT = ph_d.tile([128, T], FP8, tag="qT", bufs=1)
                kT = ph_d.tile([128, T], FP8, tag="kT", bufs=1)
                nc.sync.dma_start(
                    qT[:].rearrange("p (r w) -> p r w", r=NCORES),
                    a2a1q_out[:, 0].rearrange("r p w -> p r w"),
                )
                nc.sync.dma_start(
                    kT[:].rearrange("p (r w) -> p r w", r=NCORES),
                    a2a1q_out[:, 1].rearrange("r p w -> p r w"),
                )
                # row width 80: 64 v + ones col + 15 zero pad (DoubleRow
                # LDWEIGHTS needs plane step % 16 == 0)
                vh_sb = [
                    ph_d.tile([128, NQT, 80], FP8, tag=f"v_h{hh}", bufs=1,
                              name=f"v_h{hh}")
                    for hh in range(2)
                ]
                for hh in range(2):
                    nc.vector.memset(vh_sb[hh][:, :, 64:65], S_QKV)
                    nc.vector.memset(vh_sb[hh][:, :, 65:80], 0.0)
                    for r in range(NCORES):
                        nc.sync.dma_start(
                            vh_sb[hh][:, ds(r * RG, RG), 0:64],
                            a2a1v_out[r].rearrange(
                                "p (rg hh dh) -> p rg hh dh", rg=RG, hh=2
                            )[:, :, hh],
                        )
                # prefetch proj + fc1 weights during attention
                nc.sync.dma_start(
                    wp_sb[:].rearrange("p a b w -> p (a b) w"),
                    w_proj.rearrange("p a b w -> p (a b) w"),
                )
                nc.sync.dma_start(wfc_sb[:], wfc_v[:])

                qT_h = [qT[ds(hh * 64, 64), :] for hh in range(2)]
                kT_h = [kT[ds(hh * 64, 64), :] for hh in range(2)]

                def s_exp(qg, hh, pr):
                    """one key-tile pair: 2 S matmuls + mask + exp ->
                    et [128, 2, 512] fp8 (plane per key-tile)."""
                    st_ps = ps_s.tile([128, 2, 512], F32, tag="s")
                    for t in range(2):
                        kt = 2 * pr + t
                        nc.tensor.matmul(
                            st_ps[:, t],
                            kT_h[hh][:, ds(kt * 128, 128)],
                            qT_h[hh][:, ds(qg * 512, 512)],
                            start=True, stop=True,
                        )
                        sl = kt - 4 * qg
                        if 0 <= sl < 4:
                            nc.vector.tensor_tensor(
                                st_ps[:, t, ds(sl * 128, 128)],
                                st_ps[:, t, ds(sl * 128, 128)],
                                ntri[:], ALU.add,
                            )
                            for z in range(sl):
                                nc.vector.tensor_scalar_add(
                                    st_ps[:, t, ds(z * 128, 128)],
                                    st_ps[:, t, ds(z * 128, 128)],
                                    -1e9,
                                )
                    et = ph_d.tile([128, 2, 512], FP8, tag="et", bufs=20,
                                   name=f"et_{qg}_{hh}_{pr}")
                    nc.scalar.activation(
                        et[:].rearrange("p a b -> p (a b)"),
                        st_ps[:].rearrange("p a b -> p (a b)"),
                        AF.Exp, bias=ln16_sb[:],
                        scale=0.125 / (S_QKV * S_QKV),
                    )
                    return et

                def pv(y_ps, hh, et, pr, npr):
                    nc.tensor.matmul(
                        y_ps[:80, :],
                        vh_sb[hh][:, ds(2 * pr, 2)],
                        et[:],
                        start=(pr == 0), stop=(pr == npr - 1),
                        perf_mode=DR,
                    )

                def send_y(qg, y_ps, hh):
                    rden = ph_d.tile([1, 512], F32, tag="rden")
                    nc.vector.reciprocal(rden[:], y_ps[64:65, :])
                    dbc = ph_d.tile([64, 512], F32, tag="dbc")
                    nc.gpsimd.partition_broadcast(dbc[:], rden[:])
                    yn = ph_d.tile([64, 512], FP8, tag="yn")
                    nc.vector.scalar_tensor_tensor(
                        yn[:], y_ps[0:64, :], S_Y, dbc[:], ALU.mult, ALU.mult
                    )
                    nc.sync.dma_start(a2a2_in[qg, hh], yn[:])

                def mk_y(qg):
                    return [
                        ps_y.tile([128, 512], F32, tag=f"y{hh}", bufs=2,
                                  name=f"y{hh}_{qg}")
                        for hh in range(2)
                    ]

                for qg in range(NCORES - 1, 1, -1):
                    npr = 2 * qg + 2
                    y_ps = mk_y(qg)
                    if qg == NCORES - 1:
                        # first dest: all S+exp first, so the PE has work
                        # while the v collective lands
                        for hh in range(2):
                            ets = [s_exp(qg, hh, pr) for pr in range(npr)]
                            for pr in range(npr):
                                pv(y_ps[hh], hh, ets[pr], pr, npr)
                            send_y(qg, y_ps[hh], hh)
                    else:
                        for hh in range(2):
                            for pr in range(npr):
                                et = s_exp(qg, hh, pr)
                                pv(y_ps[hh], hh, et, pr, npr)
                            send_y(qg, y_ps[hh], hh)
                # tail dests (tiny): all S+exp first so the PV/send chains
                # overlap instead of serializing at the end
                y_tail = {qg: mk_y(qg) for qg in (1, 0)}
                for hh in range(2):
                    ets = {
                        qg: [s_exp(qg, hh, pr) for pr in range(2 * qg + 2)]
                        for qg in (1, 0)
                    }
                    for qg in (1, 0):
                        for pr in range(2 * qg + 2):
                            pv(y_tail[qg][hh], hh, ets[qg][pr], pr, 2 * qg + 2)
                        send_y(qg, y_tail[qg][hh], hh)
                nc.gpsimd.collective_compute(
                    "AllToAll",
                    ALU.bypass,
                    ins=[a2a2_in.opt()],
                    outs=[a2a2_out.opt()],
                    replica_groups=[list(range(NCORES))],
                )

            # ============ Phase E: proj (DoubleRow fp8), LN2 =================
            with (
                tc.tile_pool(name="ph_e", bufs=2) as ph_e,
                tc.tile_pool(name="ps2", bufs=3, space="PSUM") as ps2,
                tc.tile_pool(name="pspt2", bufs=2, space="PSUM") as pspt2,
            ):
                yT = ph_e.tile([128, DT, CHUNK], FP8, tag="yT", bufs=1)
                nc.sync.dma_start(
                    yT[0:64, :, :],
                    a2a2_out[:, 0].rearrange("r p w -> p r w"),
                )
                nc.sync.dma_start(
                    yT[64:128, :, :],
                    a2a2_out[:, 1].rearrange("r p w -> p r w"),
                )

                x2_sb = persist.tile([128, RG, D], F32, tag="x2")
                h2T = persist.tile([128, DT, CHUNK], FP8, tag="h2T")
                for rg in range(RG):
                    for half in range(2):
                        ps = ps2.tile([128, 512], F32, tag="mm")
                        for dp in range(DT // 2):
                            nc.tensor.matmul(
                                ps[:],
                                yT[:, ds(2 * dp, 2), ds(rg * 128, 128)],
                                wp_sb[:, half, ds(2 * dp, 2)],
                                start=(dp == 0), stop=(dp == DT // 2 - 1),
                                perf_mode=DR,
                            )
                        tmp = ph_e.tile([128, 512], F32, tag="proj_tmp")
                        nc.vector.scalar_tensor_tensor(
                            tmp[:], ps[:], 1.0 / (S_Y * S_W),
                            bproj[:, ds(half * 512, 512)], ALU.mult, ALU.add,
                        )
                        nc.vector.tensor_tensor(
                            x2_sb[:, rg, ds(half * 512, 512)], tmp[:],
                            x_sb[:, rg, ds(half * 512, 512)], ALU.add,
                        )
                # LN2 after all proj matmuls: keeps the PE dense (the
                # transposes then never wait on the DVE/ACT stat chains)
                for rg in range(RG):
                    layernorm(ph_e, pspt2, x2_sb, h2T, rg, S_H, FP8)

            # ============ Phase F: MLP (bf16), output ========================
            with (
                tc.tile_pool(name="ph_f", bufs=2) as ph_f,
                tc.tile_pool(name="ps3", bufs=3, space="PSUM") as ps3,
                tc.tile_pool(name="ps_acc", bufs=1, space="PSUM") as ps_acc,
            ):
                mT = ph_f.tile([128, NFFT, CHUNK], BF16, tag="mT", bufs=1)
                for ft in range(NFFT):
                    ps = ps3.tile([128, 512], F32, tag="mm")
                    for dp in range(DT // 2):
                        nc.tensor.matmul(
                            ps[:],
                            wfc_sb[:, ds(2 * dp, 2), ds(ft * 128, 128)],
                            h2T[:, ds(2 * dp, 2)],
                            start=(dp == 0), stop=(dp == DT // 2 - 1),
                            perf_mode=DR,
                        )
                    nc.scalar.activation(
                        mT[:, ft], ps[:], AF.Gelu_apprx_tanh,
                        bias=bfc_sb[:, ft : ft + 1], scale=1.0 / (S_H * S_W),
                    )

                out_sb = ph_f.tile([128, RG, D], F32, tag="out_sb", bufs=1)
                for half in range(2):
                    acc = [
                        ps_acc.tile([128, 512], F32, tag=f"ps_o{rg}",
                                    name=f"ps_o{rg}_{half}")
                        for rg in range(RG)
                    ]
                    for ft in range(NFFT):
                        w_t = ph_f.tile([128, 512], BF16, tag="w_fc2t", bufs=4)
                        nc.sync.dma_start(
                            w_t[:], wfc2_v[:, ft, ds(half * 512, 512)]
                        )
                        for rg in range(RG):
                            nc.tensor.matmul(
                                acc[rg][:], mT[:, ft, ds(rg * 128, 128)], w_t[:],
                                start=(ft == 0), stop=(ft == NFFT - 1),
                            )
                    for rg in range(RG):
                        tmp = ph_f.tile([128, 512], F32, tag="o_tmp")
                        nc.vector.tensor_tensor(
                            tmp[:], acc[rg][:], bfc2[:, ds(half * 512, 512)], ALU.add
                        )
                        nc.vector.tensor_tensor(
                            out_sb[:, rg, ds(half * 512, 512)], tmp[:],
                            x2_sb[:, rg, ds(half * 512, 512)], ALU.add,
                        )
                        nc.sync.dma_start(
                            out_v[:, rg, ds(half * 512, 512)],
                            out_sb[:, rg, ds(half * 512, 512)],
                        )

    nc.compile()
    return nc


def _prep_inputs(inputs):
    """Host-side shard + cast + LN affine folding + fp8 scaling."""
    bf = ml_dtypes.bfloat16
    f8 = mybir.dt.np(FP8)
    x = np.asarray(inputs["x"], np.float32).reshape(T, D)
    ln1s = np.asarray(inputs["ln1_scale"], np.float32)
    ln1b = np.asarray(inputs["ln1_bias"], np.float32)
    ln2s = np.asarray(inputs["ln2_scale"], np.float32)
    ln2b = np.asarray(inputs["ln2_bias"], np.float32)
    w_attn_f = np.asarray(inputs["W_attn"], np.float32)
    w_fc_f = np.asarray(inputs["W_fc"], np.float32)
    b_attn_f = np.asarray(inputs["b_attn"], np.float32) + ln1b @ w_attn_f
    b_fc_f = np.asarray(inputs["b_fc"], np.float32) + ln2b @ w_fc_f

    w_attn_s = ln1s[:, None] * w_attn_f
    w_qk = (w_attn_s[:, : 2 * D] * S_W).astype(f8)
    # v / proj weights rearranged to [p, half, dt, 512] (contiguous
    # DoubleRow planes), fp8 * S_W
    w_v = np.ascontiguousarray(
        (w_attn_s[:, 2 * D :] * S_W).astype(f8)
        .reshape(DT, 128, 2, 512).transpose(1, 2, 0, 3)
    )
    w_proj = np.ascontiguousarray(
        (np.asarray(inputs["W_proj"], np.float32) * S_W).astype(f8)
        .reshape(DT, 128, 2, 512).transpose(1, 2, 0, 3)
    )
    w_fc = (ln2s[:, None] * w_fc_f * S_W).astype(f8)
    w_fc2 = np.asarray(inputs["W_fc2"], np.float32).astype(bf)
    b_qk = np.ascontiguousarray(
        (b_attn_f[: 2 * D] * S_QKV).reshape(16, 128).T
    )
    bv_bc = np.broadcast_to(
        b_attn_f[2 * D :] * S_QKV, (128, D)
    ).copy().astype(np.float32)
    b_fc = np.ascontiguousarray(b_fc_f.reshape(NFFT, 128).T)
    bproj = np.broadcast_to(np.asarray(inputs["b_proj"], np.float32), (128, D)).copy()
    bfc2 = np.broadcast_to(np.asarray(inputs["b_fc2"], np.float32), (128, D)).copy()
    # additive causal mask for a diagonal tile of S^T (rows=keys,
    # cols=queries): invalid where key > query
    ntri = np.where(
        np.triu(np.ones((128, 128), bool)), 0.0, -1e9
    ).astype(np.float32)
    ident = np.eye(128, dtype=np.float32).astype(bf)

    shared = dict(
        w_qk=w_qk, w_v=w_v, w_proj=w_proj, w_fc=w_fc, w_fc2=w_fc2,
        b_qk=b_qk, b_fc=b_fc, bv_bc=bv_bc,
        bproj_bc=bproj, bfc2_bc=bfc2, ntri=ntri, ident=ident,
    )
    return [
        {"x_c": np.ascontiguousarray(x[c * CHUNK : (c + 1) * CHUNK]), **shared}
        for c in range(NCORES)
    ]


def kernel(**inputs) -> np.ndarray:
    if "nc" not in _cached:
        _cached["nc"] = _build_nc()
    nc = _cached["nc"]
    in_maps = _prep_inputs(inputs)
    res = bass_utils.run_bass_kernel_spmd(
        nc, in_maps, core_ids=list(range(NCORES))
    )
    out = np.concatenate(
        [res.results[c]["out_c"] for c in range(NCORES)], axis=0
    )
    return out.reshape(1, T, D).astype(np.float32)


# revision 38
# speedup vs baseline: 1.0039x; 1.0039x over previous
"""Trainium2 Bass kernel for a dense transformer block (pre-LN, causal attn).

Sharding across 8 NeuronCores:
  - sequence-sharded: LN1, QKV projection, attn-output proj, LN2, MLP
    (core c owns rows [512c, 512c+512) of T=4096)
  - head-sharded: attention itself (core c owns heads 2c, 2c+1 over all T)
  - collectives: a zero-cost barrier collective at t=0 absorbs the
    per-launch rank-arrival skew; A2A#1 is split into a qk collective
    (fired early) and a v collective that flies while attention S starts;
    A2A#2 carries sender-normalized y in transposed orientation. All
    three data collectives move fp8 payloads.

fp8 (e4m3) with DoubleRow (2 contraction-tiles per matmul, 2x stream) is
used for the QKV projection, PV, and the attention-output projection;
S runs in plain fp8; the MLP stays bf16 (fp8 there would breach the
error budget - measured on the reference inputs). Scales: h*16, W*128,
q/k/v*32, exp*16 (via +ln16 in the exp bias), y*32.

LayerNorm scale/bias are folded into the following matmul weights and
biases on the host; the device LN is bn_stats/bn_aggr + one activation.
Causal masking is applied additively (-1e9) on the S PSUM before exp.
Softmax skips max-subtraction (scores bounded by construction).
"""

import sys

for _p in ("/opt/trn_rl_repo", "/root/.axon_site/_ro/trn_rl_repo"):
    if _p not in sys.path:
        sys.path.insert(0, _p)

import numpy as np
import ml_dtypes

import concourse.bass as bass
import concourse.mybir as mybir
import concourse.tile as tile
from concourse import bacc, bass_utils
from concourse.bass import ds, ts

F32 = mybir.dt.float32
BF16 = mybir.dt.bfloat16
FP8 = mybir.dt.float8e4
DR = mybir.MatmulPerfMode.DoubleRow
AF = mybir.ActivationFunctionType
ALU = mybir.AluOpType

# model dims
D = 1024
T = 4096
H = 16
DH = 64
DFF = 4096
EPS = 1e-5
NCORES = 8
CHUNK = T // NCORES        # 512 rows per core
RG = CHUNK // 128          # 4 row groups
NQT = T // 128             # 32 key subblocks (for the 2 owned heads)
DT = D // 128              # 8 d-tiles
NFFT = DFF // 128          # 32 dff tiles

S_H = 16.0                 # LN1 output scale
S_W = 128.0                # fp8 weight scale
S_QKV = 32.0               # q/k/v scale
S_E = 16.0                 # exp scale (via +ln(S_E) in exp bias)
S_Y = 32.0                 # normalized-y scale

_cached = {}


def _build_nc():
    nc = bacc.Bacc("TRN2", target_bir_lowering=False)

    x_c = nc.dram_tensor("x_c", [CHUNK, D], F32, kind="ExternalInput")
    w_qk = nc.dram_tensor("w_qk", [D, 2 * D], FP8, kind="ExternalInput")
    w_v = nc.dram_tensor("w_v", [128, 2, DT, 512], FP8, kind="ExternalInput")
    w_proj = nc.dram_tensor("w_proj", [128, 2, DT, 512], FP8, kind="ExternalInput")
    w_fc = nc.dram_tensor("w_fc", [D, DFF], FP8, kind="ExternalInput")
    w_fc2 = nc.dram_tensor("w_fc2", [DFF, D], BF16, kind="ExternalInput")
    b_qk = nc.dram_tensor("b_qk", [128, 16], F32, kind="ExternalInput")
    b_fc = nc.dram_tensor("b_fc", [128, NFFT], F32, kind="ExternalInput")
    bv_bc = nc.dram_tensor("bv_bc", [128, D], F32, kind="ExternalInput")
    bproj_bc = nc.dram_tensor("bproj_bc", [128, D], F32, kind="ExternalInput")
    bfc2_bc = nc.dram_tensor("bfc2_bc", [128, D], F32, kind="ExternalInput")
    ntri_in = nc.dram_tensor("ntri", [128, 128], F32, kind="ExternalInput")
    ident_in = nc.dram_tensor("ident", [128, 128], BF16, kind="ExternalInput")

    out_c = nc.dram_tensor("out_c", [CHUNK, D], F32, kind="ExternalOutput")

    x_v = x_c.rearrange("(rg p) d -> p rg d", p=128)
    out_v = out_c.rearrange("(rg p) d -> p rg d", p=128)
    wqk_v = w_qk.rearrange("(dt p) c -> p dt c", p=128)
    wfc_v = w_fc.rearrange("(dt p) c -> p dt c", p=128)
    wfc2_v = w_fc2.rearrange("(ft p) c -> p ft c", p=128)

    with tile.TileContext(nc) as tc:
        with (
            tc.tile_pool(name="const", bufs=1) as const,
            tc.tile_pool(name="persist", bufs=1) as persist,
            tc.tile_pool(name="dram", bufs=1, space="DRAM") as dram,
        ):
            # barrier: absorb the per-launch rank skew while the PE does
            # LN1/QKV, so the first real collective doesn't pay it
            bar_in = dram.tile([NCORES, 128], BF16)
            bar_out = dram.tile([NCORES, 128], BF16)
            nc.gpsimd.collective_compute(
                "AllToAll",
                ALU.bypass,
                ins=[bar_in.opt()],
                outs=[bar_out.opt()],
                replica_groups=[list(range(NCORES))],
            )

            x_sb = persist.tile([128, RG, D], F32, tag="x_sb")
            ident = const.tile([128, 128], BF16)
            nc.sync.dma_start(x_sb[:, 0], x_v[:, 0])
            nc.sync.dma_start(ident[:], ident_in[:])
            for rg in range(1, RG):
                nc.sync.dma_start(x_sb[:, rg], x_v[:, rg])
            ntri = const.tile([128, 128], F32)
            nc.sync.dma_start(ntri[:], ntri_in[:])
            bv = const.tile([128, D], F32)
            bqk_sb = const.tile([128, 16], F32)
            bfc_sb = const.tile([128, NFFT], F32)
            bproj = const.tile([128, D], F32)
            bfc2 = const.tile([128, D], F32)
            eps_sb = const.tile([128, 1], F32)
            ln16_sb = const.tile([128, 1], F32)
            nc.vector.memset(eps_sb[:], EPS)
            nc.vector.memset(ln16_sb[:], float(np.log(S_E)))
            nc.sync.dma_start(bv[:], bv_bc[:])
            nc.sync.dma_start(bqk_sb[:], b_qk[:])

            def layernorm(pool, pt_pool, src_col, hT, rg, out_scale, dtype):
                """one row-group of LN: src [128, D] f32 -> hT cols,
                normalized only (affine folded into weights), scaled by
                out_scale, cast to dtype at the PSUM->SBUF copy."""
                xin = src_col[:, rg]
                st6 = pool.tile([128, 2, 6], F32, tag="ln_st6")
                for g in range(2):
                    nc.vector.bn_stats(st6[:, g], xin[:, ds(g * 512, 512)])
                mv = pool.tile([128, 2], F32, tag="ln_mv")
                nc.vector.bn_aggr(mv[:], st6[:])
                std = pool.tile([128, 1], F32, tag="ln_std")
                nc.scalar.activation(std[:], mv[:, 1:2], AF.Sqrt, bias=eps_sb[:])
                rstd = pool.tile([128, 1], F32, tag="ln_rstd")
                nc.vector.reciprocal(rstd[:], std[:])
                nmr = pool.tile([128, 1], F32, tag="ln_nmr")
                nc.vector.scalar_tensor_tensor(
                    nmr[:], mv[:, 0:1], -1.0, rstd[:], ALU.mult, ALU.mult
                )
                h = pool.tile([128, D], BF16, tag="ln_h")
                nc.scalar.activation(
                    h[:], xin, AF.Identity, bias=nmr[:], scale=rstd[:]
                )
                pt = pt_pool.tile([128, 1024], BF16, tag="pt")
                for d in range(DT):
                    nc.tensor.transpose(pt[:, ts(d, 128)], h[:, ts(d, 128)], ident[:])
                nc.scalar.activation(
                    hT[:, :, ds(rg * 128, 128)],
                    pt[:].rearrange("p (a b) -> p a b", a=DT),
                    AF.Copy, scale=out_scale,
                )

            # ================= Phase A/B: LN1, QK -> A2A#1qk, V -> A2A#1v ====
            a2a1q_in = dram.tile([NCORES, 2, 128, 512], FP8)
            a2a1q_out = dram.tile([NCORES, 2, 128, 512], FP8)
            a2a1v_in = dram.tile([NCORES, 128, 512], FP8)
            a2a1v_out = dram.tile([NCORES, 128, 512], FP8)
            with (
                tc.tile_pool(name="ph_a", bufs=2) as ph_a,
                tc.tile_pool(name="ps1", bufs=3, space="PSUM") as ps1,
                tc.tile_pool(name="pspt1", bufs=2, space="PSUM") as pspt1,
            ):
                hT = ph_a.tile([128, DT, CHUNK], FP8, tag="hT", bufs=1)
                wqk_sb = ph_a.tile([128, DT, 2 * D], FP8, tag="wqk_sb", bufs=1)
                nc.sync.dma_start(wqk_sb[:], wqk_v[:])
                for rg in range(RG):
                    layernorm(ph_a, pspt1, x_sb, hT, rg, S_H, FP8)

                # q/k: DoubleRow over d-tile pairs; out fp8 * S_QKV.
                # staging DMAs issue from the Activation engine so the SP
                # DMA queue never blocks the PE's weight feed.
                qkT = ph_a.tile([128, 16, CHUNK], FP8, tag="qkT", bufs=1)
                for o in range(16):
                    ps = ps1.tile([128, 512], F32, tag="mm")
                    for dp in range(DT // 2):
                        nc.tensor.matmul(
                            ps[:],
                            wqk_sb[:, ds(2 * dp, 2), ds(o * 128, 128)],
                            hT[:, ds(2 * dp, 2)],
                            start=(dp == 0), stop=(dp == DT // 2 - 1),
                            perf_mode=DR,
                        )
                    nc.scalar.activation(
                        qkT[:, o], ps[:], AF.Identity,
                        bias=bqk_sb[:, o : o + 1], scale=S_QKV / (S_H * S_W),
                    )
                    nc.scalar.dma_start(a2a1q_in[o % 8, o // 8], qkT[:, o])
                nc.gpsimd.collective_compute(
                    "AllToAll",
                    ALU.bypass,
                    ins=[a2a1q_in.opt()],
                    outs=[a2a1q_out.opt()],
                    replica_groups=[list(range(NCORES))],
                )
                # late-needed consts: keep the DMA queue clear early on
                nc.sync.dma_start(bfc_sb[:], b_fc[:])
                nc.sync.dma_start(bproj[:], bproj_bc[:])
                nc.sync.dma_start(bfc2[:], bfc2_bc[:])

                # v while the qk collective flies
                v_nat = ph_a.tile([128, RG, D], FP8, tag="v_nat", bufs=1)
                wv_t = ph_a.tile([128, 2, DT, 512], FP8, tag="w_vt", bufs=1)
                nc.sync.dma_start(
                    wv_t[:].rearrange("p a b w -> p (a b) w"),
                    w_v.rearrange("p a b w -> p (a b) w"),
                )
                for rg in range(RG):
                    for vh in range(2):
                        ps = ps1.tile([128, 512], F32, tag="mm")
                        for dp in range(DT // 2):
                            nc.tensor.matmul(
                                ps[:],
                                hT[:, ds(2 * dp, 2), ds(rg * 128, 128)],
                                wv_t[:, vh, ds(2 * dp, 2)],
                                start=(dp == 0), stop=(dp == DT // 2 - 1),
                                perf_mode=DR,
                            )
                        nc.vector.scalar_tensor_tensor(
                            v_nat[:, rg, ds(vh * 512, 512)], ps[:],
                            S_QKV / (S_H * S_W), bv[:, ds(vh * 512, 512)],
                            ALU.mult, ALU.add,
                        )
                for j in range(NCORES):
                    nc.sync.dma_start(
                        a2a1v_in[j].rearrange("p (rg w) -> p rg w", rg=RG),
                        v_nat[:, :, ds(j * 128, 128)],
                    )
                nc.gpsimd.collective_compute(
                    "AllToAll",
                    ALU.bypass,
                    ins=[a2a1v_in.opt()],
                    outs=[a2a1v_out.opt()],
                    replica_groups=[list(range(NCORES))],
                )

            # ============ Phase D: attention (2 owned heads, all T) ==========
            # Per dest chunk qg (descending, big first), per head, key-tile
            # PAIRS: two S matmuls (plain fp8) -> additive causal mask on
            # PSUM -> one exp (out fp8 * S_E) -> one DoubleRow PV matmul.
            # Sender-side normalize (reciprocal + partition broadcast).
            a2a2_in = dram.tile([NCORES, 2, 64, 512], FP8)
            a2a2_out = dram.tile([NCORES, 2, 64, 512], FP8)
            wp_sb = persist.tile([128, 2, DT, 512], FP8, tag="wp_sb")
            wfc_sb = persist.tile([128, DT, DFF], FP8, tag="wfc_sb")
            with (
                tc.tile_pool(name="ph_d", bufs=2) as ph_d,
                tc.tile_pool(name="ps_s", bufs=2, space="PSUM") as ps_s,
                tc.tile_pool(name="ps_y", bufs=2, space="PSUM") as ps_y,
            ):
                qT = ph_d.tile([128, T], FP8, tag="qT", bufs=1)
                kT = ph_d.tile([128, T], FP8, tag="kT", bufs=1)
                nc.sync.dma_start(
                    qT[:].rearrange("p (r w) -> p r w", r=NCORES),
                    a2a1q_out[:, 0].rearrange("r p w -> p r w"),
                )
                nc.sync.dma_start(
                    kT[:].rearrange("p (r w) -> p r w", r=NCORES),
                    a2a1q_out[:, 1].rearrange("r p w -> p r w"),
                )
                # row width 80: 64 v + ones col + 15 zero pad (DoubleRow
                # LDWEIGHTS needs plane step % 16 == 0)
                vh_sb = [
                    ph_d.tile([128, NQT, 80], FP8, tag=f"v_h{hh}", bufs=1,
                              name=f"v_h{hh}")
                    for hh in range(2)
                ]
                for hh in range(2):
                    nc.vector.memset(vh_sb[hh][:, :, 64:65], S_QKV)
                    nc.vector.memset(vh_sb[hh][:, :, 65:80], 0.0)
                    for r in range(NCORES):
                        nc.sync.dma_start(
                            vh_sb[hh][:, ds(r * RG, RG), 0:64],
                            a2a1v_out[r].rearrange(
                                "p (rg hh dh) -> p rg hh dh", rg=RG, hh=2
                            )[:, :, hh],
                        )
                # prefetch proj + fc1 weights during attention
                nc.sync.dma_start(
                    wp_sb[:].rearrange("p a b w -> p (a b) w"),
                    w_proj.rearrange("p a b w -> p (a b) w"),
                )
                nc.sync.dma_start(wfc_sb[:], wfc_v[:])

                qT_h = [qT[ds(hh * 64, 64), :] for hh in range(2)]
                kT_h = [kT[ds(hh * 64, 64), :] for hh in range(2)]

                def s_exp(qg, hh, pr):
                    """one key-tile pair: 2 S matmuls + mask + exp ->
                    et [128, 2, 512] fp8 (plane per key-tile)."""
                    st_ps = ps_s.tile([128, 2, 512], F32, tag="s")
                    for t in range(2):
                        kt = 2 * pr + t
                        nc.tensor.matmul(
                            st_ps[:, t],
                            kT_h[hh][:, ds(kt * 128, 128)],
                            qT_h[hh][:, ds(qg * 512, 512)],
                            start=True, stop=True,
                        )
                        sl = kt - 4 * qg
                        if 0 <= sl < 4:
                            nc.vector.tensor_tensor(
                                st_ps[:, t, ds(sl * 128, 128)],
                                st_ps[:, t, ds(sl * 128, 128)],
                                ntri[:], ALU.add,
                            )
                            for z in range(sl):
                                nc.vector.tensor_scalar_add(
                                    st_ps[:, t, ds(z * 128, 128)],
                                    st_ps[:, t, ds(z * 128, 128)],
                                    -1e9,
                                )
                    et = ph_d.tile([128, 2, 512], FP8, tag="et", bufs=20,
                                   name=f"et_{qg}_{hh}_{pr}")
                    nc.scalar.activation(
                        et[:].rearrange("p a b -> p (a b)"),
                        st_ps[:].rearrange("p a b -> p (a b)"),
                        AF.Exp, bias=ln16_sb[:],
                        scale=0.125 / (S_QKV * S_QKV),
                    )
                    return et

                def pv(y_ps, hh, et, pr, npr):
                    nc.tensor.matmul(
                        y_ps[:80, :],
                        vh_sb[hh][:, ds(2 * pr, 2)],
                        et[:],
                        start=(pr == 0), stop=(pr == npr - 1),
                        perf_mode=DR,
                    )

                def send_y(qg, y_ps, hh):
                    rden = ph_d.tile([1, 512], F32, tag="rden")
                    nc.vector.reciprocal(rden[:], y_ps[64:65, :])
                    dbc = ph_d.tile([64, 512], F32, tag="dbc")
                    nc.gpsimd.partition_broadcast(dbc[:], rden[:])
                    yn = ph_d.tile([64, 512], FP8, tag="yn")
                    nc.vector.scalar_tensor_tensor(
                        yn[:], y_ps[0:64, :], S_Y, dbc[:], ALU.mult, ALU.mult
                    )
                    nc.sync.dma_start(a2a2_in[qg, hh], yn[:])

                def mk_y(qg):
                    return [
                        ps_y.tile([128, 512], F32, tag=f"y{hh}", bufs=2,
                                  name=f"y{hh}_{qg}")
                        for hh in range(2)
                    ]

                for qg in range(NCORES - 1, 1, -1):
                    npr = 2 * qg + 2
                    y_ps = mk_y(qg)
                    if qg == NCORES - 1:
                        # first dest: all S+exp first, so the PE has work
                        # while the v collective lands
                        for hh in range(2):
                            ets = [s_exp(qg, hh, pr) for pr in range(npr)]
                            for pr in range(npr):
                                pv(y_ps[hh], hh, ets[pr], pr, npr)
                            send_y(qg, y_ps[hh], hh)
                    else:
                        for hh in range(2):
                            for pr in range(npr):
                                et = s_exp(qg, hh, pr)
                                pv(y_ps[hh], hh, et, pr, npr)
                            send_y(qg, y_ps[hh], hh)
                # tail dests (tiny): all S+exp first so the PV/send chains
                # overlap instead of serializing at the end
                y_tail = {qg: mk_y(qg) for qg in (1, 0)}
                for hh in range(2):
                    ets = {
                        qg: [s_exp(qg, hh, pr) for pr in range(2 * qg + 2)]
                        for qg in (1, 0)
                    }
                    for qg in (1, 0):
                        for pr in range(2 * qg + 2):
                            pv(y_tail[qg][hh], hh, ets[qg][pr], pr, 2 * qg + 2)
                        send_y(qg, y_tail[qg][hh], hh)
                nc.gpsimd.collective_compute(
                    "AllToAll",
                    ALU.bypass,
                    ins=[a2a2_in.opt()],
                    outs=[a2a2_out.opt()],
                    replica_groups=[list(range(NCORES))],
                )

            # ============ Phase E: proj (DoubleRow fp8), LN2 =================
            with (
                tc.tile_pool(name="ph_e", bufs=2) as ph_e,
                tc.tile_pool(name="ps2", bufs=3, space="PSUM") as ps2,
                tc.tile_pool(name="pspt2", bufs=2, space="PSUM") as pspt2,
            ):
                yT = ph_e.tile([128, DT, CHUNK], FP8, tag="yT", bufs=1)
                nc.sync.dma_start(
                    yT[0:64, :, :],
                    a2a2_out[:, 0].rearrange("r p w -> p r w"),
                )
                nc.sync.dma_start(
                    yT[64:128, :, :],
                    a2a2_out[:, 1].rearrange("r p w -> p r w"),
                )

                x2_sb = persist.tile([128, RG, D], F32, tag="x2")
                h2T = persist.tile([128, DT, CHUNK], FP8, tag="h2T")
                for rg in range(RG):
                    for half in range(2):
                        ps = ps2.tile([128, 512], F32, tag="mm")
                        for dp in range(DT // 2):
                            nc.tensor.matmul(
                                ps[:],
                                yT[:, ds(2 * dp, 2), ds(rg * 128, 128)],
                                wp_sb[:, half, ds(2 * dp, 2)],
                                start=(dp == 0), stop=(dp == DT // 2 - 1),
                                perf_mode=DR,
                            )
                        tmp = ph_e.tile([128, 512], F32, tag="proj_tmp")
                        nc.vector.scalar_tensor_tensor(
                            tmp[:], ps[:], 1.0 / (S_Y * S_W),
                            bproj[:, ds(half * 512, 512)], ALU.mult, ALU.add,
                        )
                        nc.vector.tensor_tensor(
                            x2_sb[:, rg, ds(half * 512, 512)], tmp[:],
                            x_sb[:, rg, ds(half * 512, 512)], ALU.add,
                        )
                    layernorm(ph_e, pspt2, x2_sb, h2T, rg, S_H, FP8)

            # ============ Phase F: MLP (bf16), output ========================
            with (
                tc.tile_pool(name="ph_f", bufs=2) as ph_f,
                tc.tile_pool(name="ps3", bufs=3, space="PSUM") as ps3,
                tc.tile_pool(name="ps_acc", bufs=1, space="PSUM") as ps_acc,
            ):
                mT = ph_f.tile([128, NFFT, CHUNK], BF16, tag="mT", bufs=1)
                for ft in range(NFFT):
                    ps = ps3.tile([128, 512], F32, tag="mm")
                    for dp in range(DT // 2):
                        nc.tensor.matmul(
                            ps[:],
                            wfc_sb[:, ds(2 * dp, 2), ds(ft * 128, 128)],
                            h2T[:, ds(2 * dp, 2)],
                            start=(dp == 0), stop=(dp == DT // 2 - 1),
                            perf_mode=DR,
                        )
                    nc.scalar.activation(
                        mT[:, ft], ps[:], AF.Gelu_apprx_tanh,
                        bias=bfc_sb[:, ft : ft + 1], scale=1.0 / (S_H * S_W),
                    )

                out_sb = ph_f.tile([128, RG, D], F32, tag="out_sb", bufs=1)
                for half in range(2):
                    acc = [
                        ps_acc.tile([128, 512], F32, tag=f"ps_o{rg}",
                                    name=f"ps_o{rg}_{half}")
                        for rg in range(RG)
                    ]
                    for ft in range(NFFT):
                        w_t = ph_f.tile([128, 512], BF16, tag="w_fc2t", bufs=4)
                        nc.sync.dma_start(
                            w_t[:], wfc2_v[:, ft, ds(half * 512, 512)]
                        )
                        for rg in range(RG):
                            nc.tensor.matmul(
                                acc[rg][:], mT[:, ft, ds(rg * 128, 128)], w_t[:],
                                start=(ft == 0), stop=(ft == NFFT - 1),
                            )
                    for rg in range(RG):
                        tmp = ph_f.tile([128, 512], F32, tag="o_tmp")
                        nc.vector.tensor_tensor(
                            tmp[:], acc[rg][:], bfc2[:, ds(half * 512, 512)], ALU.add
                        )
                        nc.vector.tensor_tensor(
                            out_sb[:, rg, ds(half * 512, 512)], tmp[:],
                            x2_sb[:, rg, ds(half * 512, 512)], ALU.add,
                        )
                        nc.sync.dma_start(
                            out_v[:, rg, ds(half * 512, 512)],
                            out_sb[:, rg, ds(half * 512, 512)],
                        )

    nc.compile()
    return nc


def _prep_inputs(inputs):
    """Host-side shard + cast + LN affine folding + fp8 scaling."""
    bf = ml_dtypes.bfloat16
    f8 = mybir.dt.np(FP8)
    x = np.asarray(inputs["x"], np.float32).reshape(T, D)
    ln1s = np.asarray(inputs["ln1_scale"], np.float32)
    ln1b = np.asarray(inputs["ln1_bias"], np.float32)
    ln2s = np.asarray(inputs["ln2_scale"], np.float32)
    ln2b = np.asarray(inputs["ln2_bias"], np.float32)
    w_attn_f = np.asarray(inputs["W_attn"], np.float32)
    w_fc_f = np.asarray(inputs["W_fc"], np.float32)
    b_attn_f = np.asarray(inputs["b_attn"], np.float32) + ln1b @ w_attn_f
    b_fc_f = np.asarray(inputs["b_fc"], np.float32) + ln2b @ w_fc_f

    w_attn_s = ln1s[:, None] * w_attn_f
    w_qk = (w_attn_s[:, : 2 * D] * S_W).astype(f8)
    # v / proj weights rearranged to [p, half, dt, 512] (contiguous
    # DoubleRow planes), fp8 * S_W
    w_v = np.ascontiguousarray(
        (w_attn_s[:, 2 * D :] * S_W).astype(f8)
        .reshape(DT, 128, 2, 512).transpose(1, 2, 0, 3)
    )
    w_proj = np.ascontiguousarray(
        (np.asarray(inputs["W_proj"], np.float32) * S_W).astype(f8)
        .reshape(DT, 128, 2, 512).transpose(1, 2, 0, 3)
    )
    w_fc = (ln2s[:, None] * w_fc_f * S_W).astype(f8)
    w_fc2 = np.asarray(inputs["W_fc2"], np.float32).astype(bf)
    b_qk = np.ascontiguousarray(
        (b_attn_f[: 2 * D] * S_QKV).reshape(16, 128).T
    )
    bv_bc = np.broadcast_to(
        b_attn_f[2 * D :] * S_QKV, (128, D)
    ).copy().astype(np.float32)
    b_fc = np.ascontiguousarray(b_fc_f.reshape(NFFT, 128).T)
    bproj = np.broadcast_to(np.asarray(inputs["b_proj"], np.float32), (128, D)).copy()
    bfc2 = np.broadcast_to(np.asarray(inputs["b_fc2"], np.float32), (128, D)).copy()
    # additive causal mask for a diagonal tile of S^T (rows=keys,
    # cols=queries): invalid where key > query
    ntri = np.where(
        np.triu(np.ones((128, 128), bool)), 0.0, -1e9
    ).astype(np.float32)
    ident = np.eye(128, dtype=np.float32).astype(bf)

    shared = dict(
        w_qk=w_qk, w_v=w_v, w_proj=w_proj, w_fc=w_fc, w_fc2=w_fc2,
        b_qk=b_qk, b_fc=b_fc, bv_bc=bv_bc,
        bproj_bc=bproj, bfc2_bc=bfc2, ntri=ntri, ident=ident,
    )
    return [
        {"x_c": np.ascontiguousarray(x[c * CHUNK : (c + 1) * CHUNK]), **shared}
        for c in range(NCORES)
    ]


def kernel(**inputs) -> np.ndarray:
    if "nc" not in _cached:
        _cached["nc"] = _build_nc()
    nc = _cached["nc"]
    in_maps = _prep_inputs(inputs)
    res = bass_utils.run_bass_kernel_spmd(
        nc, in_maps, core_ids=list(range(NCORES))
    )
    out = np.concatenate(
        [res.results[c]["out_c"] for c in range(NCORES)], axis=0
    )
    return out.reshape(1, T, D).astype(np.float32)


# revision 40
# speedup vs baseline: 1.0317x; 1.0277x over previous
"""Trainium2 Bass kernel for a dense transformer block (pre-LN, causal attn).

Sharding across 8 NeuronCores:
  - sequence-sharded: LN1, QKV projection, attn-output proj, LN2, MLP
    (core c owns rows [512c, 512c+512) of T=4096)
  - head-sharded: attention itself (core c owns heads 2c, 2c+1 over all T)
  - collectives: a zero-cost barrier collective at t=0 absorbs the
    per-launch rank-arrival skew; A2A#1 is split into a qk collective
    (fired early) and a v collective that flies while attention S starts;
    A2A#2 carries sender-normalized y in transposed orientation. All
    three data collectives move fp8 payloads.

fp8 (e4m3) with DoubleRow (2 contraction-tiles per matmul, 2x stream) is
used for the QKV projection, PV, and the attention-output projection;
S runs in plain fp8; the MLP stays bf16 (fp8 there would breach the
error budget - measured on the reference inputs). Scales: h*16, W*128,
q/k/v*32, exp*16 (via +ln16 in the exp bias), y*32.

LayerNorm scale/bias are folded into the following matmul weights and
biases on the host; the device LN is bn_stats/bn_aggr + one activation.
Causal masking is applied additively (-1e9) on the S PSUM before exp.
Softmax skips max-subtraction (scores bounded by construction).
"""

import sys

for _p in ("/opt/trn_rl_repo", "/root/.axon_site/_ro/trn_rl_repo"):
    if _p not in sys.path:
        sys.path.insert(0, _p)

import numpy as np
import ml_dtypes

import concourse.bass as bass
import concourse.mybir as mybir
import concourse.tile as tile
from concourse import bacc, bass_utils
from concourse.bass import ds, ts

F32 = mybir.dt.float32
BF16 = mybir.dt.bfloat16
FP8 = mybir.dt.float8e4
DR = mybir.MatmulPerfMode.DoubleRow
AF = mybir.ActivationFunctionType
ALU = mybir.AluOpType

# model dims
D = 1024
T = 4096
H = 16
DH = 64
DFF = 4096
EPS = 1e-5
NCORES = 8
CHUNK = T // NCORES        # 512 rows per core
RG = CHUNK // 128          # 4 row groups
NQT = T // 128             # 32 key subblocks (for the 2 owned heads)
DT = D // 128              # 8 d-tiles
NFFT = DFF // 128          # 32 dff tiles

S_H = 16.0                 # LN1 output scale
S_W = 128.0                # fp8 weight scale
S_QKV = 32.0               # q/k/v scale
S_E = 16.0                 # exp scale (via +ln(S_E) in exp bias)
S_Y = 32.0                 # normalized-y scale

_cached = {}


def _build_nc():
    nc = bacc.Bacc("TRN2", target_bir_lowering=False)

    x_c = nc.dram_tensor("x_c", [CHUNK, D], F32, kind="ExternalInput")
    w_qk = nc.dram_tensor("w_qk", [D, 2 * D], FP8, kind="ExternalInput")
    w_v = nc.dram_tensor("w_v", [128, 2, DT, 512], FP8, kind="ExternalInput")
    w_proj = nc.dram_tensor("w_proj", [128, 2, DT, 512], FP8, kind="ExternalInput")
    w_fc = nc.dram_tensor("w_fc", [D, DFF], FP8, kind="ExternalInput")
    w_fc2 = nc.dram_tensor("w_fc2", [DFF, D], BF16, kind="ExternalInput")
    b_qk = nc.dram_tensor("b_qk", [128, 16], F32, kind="ExternalInput")
    b_fc = nc.dram_tensor("b_fc", [128, NFFT], F32, kind="ExternalInput")
    bv_bc = nc.dram_tensor("bv_bc", [128, D], F32, kind="ExternalInput")
    bproj_bc = nc.dram_tensor("bproj_bc", [128, D], F32, kind="ExternalInput")
    bfc2_bc = nc.dram_tensor("bfc2_bc", [128, D], F32, kind="ExternalInput")
    ntri_in = nc.dram_tensor("ntri", [128, 128], F32, kind="ExternalInput")
    ident_in = nc.dram_tensor("ident", [128, 128], BF16, kind="ExternalInput")

    out_c = nc.dram_tensor("out_c", [CHUNK, D], F32, kind="ExternalOutput")

    x_v = x_c.rearrange("(rg p) d -> p rg d", p=128)
    out_v = out_c.rearrange("(rg p) d -> p rg d", p=128)
    wqk_v = w_qk.rearrange("(dt p) c -> p dt c", p=128)
    wfc_v = w_fc.rearrange("(dt p) c -> p dt c", p=128)
    wfc2_v = w_fc2.rearrange("(ft p) c -> p ft c", p=128)

    with tile.TileContext(nc) as tc:
        with (
            tc.tile_pool(name="const", bufs=1) as const,
            tc.tile_pool(name="persist", bufs=1) as persist,
            tc.tile_pool(name="dram", bufs=1, space="DRAM") as dram,
        ):
            # barrier: absorb the per-launch rank skew while the PE does
            # LN1/QKV, so the first real collective doesn't pay it
            bar_in = dram.tile([NCORES, 128], BF16)
            bar_out = dram.tile([NCORES, 128], BF16)
            nc.gpsimd.collective_compute(
                "AllToAll",
                ALU.bypass,
                ins=[bar_in.opt()],
                outs=[bar_out.opt()],
                replica_groups=[list(range(NCORES))],
            )

            x_sb = persist.tile([128, RG, D], F32, tag="x_sb")
            ident = const.tile([128, 128], BF16)
            nc.sync.dma_start(x_sb[:, 0], x_v[:, 0])
            nc.sync.dma_start(ident[:], ident_in[:])
            for rg in range(1, RG):
                nc.sync.dma_start(x_sb[:, rg], x_v[:, rg])
            ntri = const.tile([128, 128], F32)
            nc.sync.dma_start(ntri[:], ntri_in[:])
            bv = const.tile([128, D], F32)
            bqk_sb = const.tile([128, 16], F32)
            bfc_sb = const.tile([128, NFFT], F32)
            bproj = const.tile([128, D], F32)
            bfc2 = const.tile([128, D], F32)
            eps_sb = const.tile([128, 1], F32)
            ln16_sb = const.tile([128, 1], F32)
            nc.vector.memset(eps_sb[:], EPS)
            nc.vector.memset(ln16_sb[:], float(np.log(S_E)))
            nc.sync.dma_start(bv[:], bv_bc[:])
            nc.sync.dma_start(bqk_sb[:], b_qk[:])

            def layernorm(pool, pt_pool, src_col, hT, rg, out_scale, dtype):
                """one row-group of LN: src [128, D] f32 -> hT cols,
                normalized only (affine folded into weights), scaled by
                out_scale, cast to dtype at the PSUM->SBUF copy."""
                xin = src_col[:, rg]
                st6 = pool.tile([128, 2, 6], F32, tag="ln_st6")
                for g in range(2):
                    nc.vector.bn_stats(st6[:, g], xin[:, ds(g * 512, 512)])
                mv = pool.tile([128, 2], F32, tag="ln_mv")
                nc.vector.bn_aggr(mv[:], st6[:])
                std = pool.tile([128, 1], F32, tag="ln_std")
                nc.scalar.activation(std[:], mv[:, 1:2], AF.Sqrt, bias=eps_sb[:])
                rstd = pool.tile([128, 1], F32, tag="ln_rstd")
                nc.vector.reciprocal(rstd[:], std[:])
                nmr = pool.tile([128, 1], F32, tag="ln_nmr")
                nc.vector.scalar_tensor_tensor(
                    nmr[:], mv[:, 0:1], -1.0, rstd[:], ALU.mult, ALU.mult
                )
                h = pool.tile([128, D], BF16, tag="ln_h")
                nc.scalar.activation(
                    h[:], xin, AF.Identity, bias=nmr[:], scale=rstd[:]
                )
                pt = pt_pool.tile([128, 1024], BF16, tag="pt")
                for d in range(DT):
                    nc.tensor.transpose(pt[:, ts(d, 128)], h[:, ts(d, 128)], ident[:])
                nc.scalar.activation(
                    hT[:, :, ds(rg * 128, 128)],
                    pt[:].rearrange("p (a b) -> p a b", a=DT),
                    AF.Copy, scale=out_scale,
                )

            # ================= Phase A/B: LN1, QK -> A2A#1qk, V -> A2A#1v ====
            a2a1q_in = dram.tile([NCORES, 2, 128, 512], FP8)
            a2a1q_out = dram.tile([NCORES, 2, 128, 512], FP8)
            a2a1v_in = dram.tile([NCORES, 128, 512], FP8)
            a2a1v_out = dram.tile([NCORES, 128, 512], FP8)
            with (
                tc.tile_pool(name="ph_a", bufs=2) as ph_a,
                tc.tile_pool(name="ps1", bufs=3, space="PSUM") as ps1,
                tc.tile_pool(name="pspt1", bufs=2, space="PSUM") as pspt1,
            ):
                hT = ph_a.tile([128, DT, CHUNK], FP8, tag="hT", bufs=1)
                wqk_sb = ph_a.tile([128, DT, 2 * D], FP8, tag="wqk_sb", bufs=1)
                nc.sync.dma_start(wqk_sb[:], wqk_v[:])
                for rg in range(RG):
                    layernorm(ph_a, pspt1, x_sb, hT, rg, S_H, FP8)

                # q/k: DoubleRow over d-tile pairs; out fp8 * S_QKV.
                # staging DMAs issue from the Activation engine so the SP
                # DMA queue never blocks the PE's weight feed.
                qkT = ph_a.tile([128, 16, CHUNK], FP8, tag="qkT", bufs=1)
                for o in range(16):
                    ps = ps1.tile([128, 512], F32, tag="mm")
                    for dp in range(DT // 2):
                        nc.tensor.matmul(
                            ps[:],
                            wqk_sb[:, ds(2 * dp, 2), ds(o * 128, 128)],
                            hT[:, ds(2 * dp, 2)],
                            start=(dp == 0), stop=(dp == DT // 2 - 1),
                            perf_mode=DR,
                        )
                    nc.scalar.activation(
                        qkT[:, o], ps[:], AF.Identity,
                        bias=bqk_sb[:, o : o + 1], scale=S_QKV / (S_H * S_W),
                    )
                    nc.scalar.dma_start(a2a1q_in[o % 8, o // 8], qkT[:, o])
                nc.gpsimd.collective_compute(
                    "AllToAll",
                    ALU.bypass,
                    ins=[a2a1q_in.opt()],
                    outs=[a2a1q_out.opt()],
                    replica_groups=[list(range(NCORES))],
                )
                # late-needed consts: keep the DMA queue clear early on
                nc.sync.dma_start(bfc_sb[:], b_fc[:])
                nc.sync.dma_start(bproj[:], bproj_bc[:])
                nc.sync.dma_start(bfc2[:], bfc2_bc[:])

                # v while the qk collective flies
                v_nat = ph_a.tile([128, RG, D], FP8, tag="v_nat", bufs=1)
                wv_t = ph_a.tile([128, 2, DT, 512], FP8, tag="w_vt", bufs=1)
                nc.sync.dma_start(
                    wv_t[:].rearrange("p a b w -> p (a b) w"),
                    w_v.rearrange("p a b w -> p (a b) w"),
                )
                for rg in range(RG):
                    for vh in range(2):
                        ps = ps1.tile([128, 512], F32, tag="mm")
                        for dp in range(DT // 2):
                            nc.tensor.matmul(
                                ps[:],
                                hT[:, ds(2 * dp, 2), ds(rg * 128, 128)],
                                wv_t[:, vh, ds(2 * dp, 2)],
                                start=(dp == 0), stop=(dp == DT // 2 - 1),
                                perf_mode=DR,
                            )
                        nc.vector.scalar_tensor_tensor(
                            v_nat[:, rg, ds(vh * 512, 512)], ps[:],
                            S_QKV / (S_H * S_W), bv[:, ds(vh * 512, 512)],
                            ALU.mult, ALU.add,
                        )
                for j in range(NCORES):
                    nc.sync.dma_start(
                        a2a1v_in[j].rearrange("p (rg w) -> p rg w", rg=RG),
                        v_nat[:, :, ds(j * 128, 128)],
                    )
                nc.gpsimd.collective_compute(
                    "AllToAll",
                    ALU.bypass,
                    ins=[a2a1v_in.opt()],
                    outs=[a2a1v_out.opt()],
                    replica_groups=[list(range(NCORES))],
                )

            # ============ Phase D: attention (2 owned heads, all T) ==========
            # Per dest chunk qg (descending, big first), per head, key-tile
            # PAIRS: two S matmuls (plain fp8) -> additive causal mask on
            # PSUM -> one exp (out fp8 * S_E) -> one DoubleRow PV matmul.
            # Sender-side normalize (reciprocal + partition broadcast).
            a2a2_in = dram.tile([NCORES, 2, 64, 512], FP8)
            a2a2_out = dram.tile([NCORES, 2, 64, 512], FP8)
            wp_sb = persist.tile([128, 2, DT, 512], FP8, tag="wp_sb")
            wfc_sb = persist.tile([128, DT, DFF], FP8, tag="wfc_sb")
            with (
                tc.tile_pool(name="ph_d", bufs=2) as ph_d,
                tc.tile_pool(name="ps_s", bufs=2, space="PSUM") as ps_s,
                tc.tile_pool(name="ps_y", bufs=2, space="PSUM") as ps_y,
            ):
                qT = ph_d.tile([128, T], FP8, tag="qT", bufs=1)
                kT = ph_d.tile([128, T], FP8, tag="kT", bufs=1)
                nc.sync.dma_start(
                    qT[:].rearrange("p (r w) -> p r w", r=NCORES),
                    a2a1q_out[:, 0].rearrange("r p w -> p r w"),
                )
                nc.sync.dma_start(
                    kT[:].rearrange("p (r w) -> p r w", r=NCORES),
                    a2a1q_out[:, 1].rearrange("r p w -> p r w"),
                )
                # row width 80: 64 v + ones col + 15 zero pad (DoubleRow
                # LDWEIGHTS needs plane step % 16 == 0)
                vh_sb = [
                    ph_d.tile([128, NQT, 80], FP8, tag=f"v_h{hh}", bufs=1,
                              name=f"v_h{hh}")
                    for hh in range(2)
                ]
                for hh in range(2):
                    nc.vector.memset(vh_sb[hh][:, :, 64:65], S_QKV)
                    nc.vector.memset(vh_sb[hh][:, :, 65:80], 0.0)
                    for r in range(NCORES):
                        nc.sync.dma_start(
                            vh_sb[hh][:, ds(r * RG, RG), 0:64],
                            a2a1v_out[r].rearrange(
                                "p (rg hh dh) -> p rg hh dh", rg=RG, hh=2
                            )[:, :, hh],
                        )
                # prefetch proj + fc1 weights during attention
                nc.sync.dma_start(
                    wp_sb[:].rearrange("p a b w -> p (a b) w"),
                    w_proj.rearrange("p a b w -> p (a b) w"),
                )
                nc.sync.dma_start(wfc_sb[:], wfc_v[:])

                qT_h = [qT[ds(hh * 64, 64), :] for hh in range(2)]
                kT_h = [kT[ds(hh * 64, 64), :] for hh in range(2)]

                def s_exp(qg, hh, pr):
                    """one key-tile pair: 2 S matmuls + mask + exp ->
                    et [128, 2, 512] fp8 (plane per key-tile)."""
                    st_ps = ps_s.tile([128, 2, 512], F32, tag="s")
                    for t in range(2):
                        kt = 2 * pr + t
                        nc.tensor.matmul(
                            st_ps[:, t],
                            kT_h[hh][:, ds(kt * 128, 128)],
                            qT_h[hh][:, ds(qg * 512, 512)],
                            start=True, stop=True,
                        )
                        sl = kt - 4 * qg
                        if 0 <= sl < 4:
                            nc.vector.tensor_tensor(
                                st_ps[:, t, ds(sl * 128, 128)],
                                st_ps[:, t, ds(sl * 128, 128)],
                                ntri[:], ALU.add,
                            )
                            for z in range(sl):
                                nc.vector.tensor_scalar_add(
                                    st_ps[:, t, ds(z * 128, 128)],
                                    st_ps[:, t, ds(z * 128, 128)],
                                    -1e9,
                                )
                    et = ph_d.tile([128, 2, 512], FP8, tag="et", bufs=36,
                                   name=f"et_{qg}_{hh}_{pr}")
                    nc.scalar.activation(
                        et[:].rearrange("p a b -> p (a b)"),
                        st_ps[:].rearrange("p a b -> p (a b)"),
                        AF.Exp, bias=ln16_sb[:],
                        scale=0.125 / (S_QKV * S_QKV),
                    )
                    return et

                def pv(y_ps, hh, et, pr, npr):
                    nc.tensor.matmul(
                        y_ps[:80, :],
                        vh_sb[hh][:, ds(2 * pr, 2)],
                        et[:],
                        start=(pr == 0), stop=(pr == npr - 1),
                        perf_mode=DR,
                    )

                def send_y(qg, y_ps, hh):
                    rden = ph_d.tile([1, 512], F32, tag="rden")
                    nc.vector.reciprocal(rden[:], y_ps[64:65, :])
                    dbc = ph_d.tile([64, 512], F32, tag="dbc")
                    nc.gpsimd.partition_broadcast(dbc[:], rden[:])
                    yn = ph_d.tile([64, 512], FP8, tag="yn")
                    nc.vector.scalar_tensor_tensor(
                        yn[:], y_ps[0:64, :], S_Y, dbc[:], ALU.mult, ALU.mult
                    )
                    nc.sync.dma_start(a2a2_in[qg, hh], yn[:])

                def mk_y(qg):
                    return [
                        ps_y.tile([128, 512], F32, tag=f"y{hh}", bufs=2,
                                  name=f"y{hh}_{qg}")
                        for hh in range(2)
                    ]

                # heads are interleaved per key-tile pair: head A's S
                # matmuls occupy PE rows 0-63, head B's rows 64-127, so the
                # PE's reorder window pulls each head's LDWEIGHTS ahead
                # under the other head's matmul stream.
                for qg in range(NCORES - 1, 1, -1):
                    npr = 2 * qg + 2
                    y_ps = mk_y(qg)
                    if qg == NCORES - 1:
                        # first dest: all S+exp first, so the PE has work
                        # while the v collective lands
                        ets = [
                            [s_exp(qg, hh, pr) for hh in range(2)]
                            for pr in range(npr)
                        ]
                        for pr in range(npr):
                            for hh in range(2):
                                pv(y_ps[hh], hh, ets[pr][hh], pr, npr)
                        for hh in range(2):
                            send_y(qg, y_ps[hh], hh)
                    else:
                        for pr in range(npr):
                            es = [s_exp(qg, hh, pr) for hh in range(2)]
                            for hh in range(2):
                                pv(y_ps[hh], hh, es[hh], pr, npr)
                        for hh in range(2):
                            send_y(qg, y_ps[hh], hh)
                # tail dests (tiny): all S+exp first so the PV/send chains
                # overlap instead of serializing at the end
                y_tail = {qg: mk_y(qg) for qg in (1, 0)}
                ets_t = {
                    qg: [
                        [s_exp(qg, hh, pr) for hh in range(2)]
                        for pr in range(2 * qg + 2)
                    ]
                    for qg in (1, 0)
                }
                for qg in (1, 0):
                    for pr in range(2 * qg + 2):
                        for hh in range(2):
                            pv(y_tail[qg][hh], hh, ets_t[qg][pr][hh],
                               pr, 2 * qg + 2)
                    for hh in range(2):
                        send_y(qg, y_tail[qg][hh], hh)
                nc.gpsimd.collective_compute(
                    "AllToAll",
                    ALU.bypass,
                    ins=[a2a2_in.opt()],
                    outs=[a2a2_out.opt()],
                    replica_groups=[list(range(NCORES))],
                )

            # ============ Phase E: proj (DoubleRow fp8), LN2 =================
            with (
                tc.tile_pool(name="ph_e", bufs=2) as ph_e,
                tc.tile_pool(name="ps2", bufs=3, space="PSUM") as ps2,
                tc.tile_pool(name="pspt2", bufs=2, space="PSUM") as pspt2,
            ):
                yT = ph_e.tile([128, DT, CHUNK], FP8, tag="yT", bufs=1)
                nc.sync.dma_start(
                    yT[0:64, :, :],
                    a2a2_out[:, 0].rearrange("r p w -> p r w"),
                )
                nc.sync.dma_start(
                    yT[64:128, :, :],
                    a2a2_out[:, 1].rearrange("r p w -> p r w"),
                )

                x2_sb = persist.tile([128, RG, D], F32, tag="x2")
                h2T = persist.tile([128, DT, CHUNK], FP8, tag="h2T")
                for rg in range(RG):
                    for half in range(2):
                        ps = ps2.tile([128, 512], F32, tag="mm")
                        for dp in range(DT // 2):
                            nc.tensor.matmul(
                                ps[:],
                                yT[:, ds(2 * dp, 2), ds(rg * 128, 128)],
                                wp_sb[:, half, ds(2 * dp, 2)],
                                start=(dp == 0), stop=(dp == DT // 2 - 1),
                                perf_mode=DR,
                            )
                        tmp = ph_e.tile([128, 512], F32, tag="proj_tmp")
                        nc.vector.scalar_tensor_tensor(
                            tmp[:], ps[:], 1.0 / (S_Y * S_W),
                            bproj[:, ds(half * 512, 512)], ALU.mult, ALU.add,
                        )
                        nc.vector.tensor_tensor(
                            x2_sb[:, rg, ds(half * 512, 512)], tmp[:],
                            x_sb[:, rg, ds(half * 512, 512)], ALU.add,
                        )
                    layernorm(ph_e, pspt2, x2_sb, h2T, rg, S_H, FP8)

            # ============ Phase F: MLP (bf16), output ========================
            with (
                tc.tile_pool(name="ph_f", bufs=2) as ph_f,
                tc.tile_pool(name="ps3", bufs=3, space="PSUM") as ps3,
                tc.tile_pool(name="ps_acc", bufs=1, space="PSUM") as ps_acc,
            ):
                mT = ph_f.tile([128, NFFT, CHUNK], BF16, tag="mT", bufs=1)
                for ft in range(NFFT):
                    ps = ps3.tile([128, 512], F32, tag="mm")
                    for dp in range(DT // 2):
                        nc.tensor.matmul(
                            ps[:],
                            wfc_sb[:, ds(2 * dp, 2), ds(ft * 128, 128)],
                            h2T[:, ds(2 * dp, 2)],
                            start=(dp == 0), stop=(dp == DT // 2 - 1),
                            perf_mode=DR,
                        )
                    nc.scalar.activation(
                        mT[:, ft], ps[:], AF.Gelu_apprx_tanh,
                        bias=bfc_sb[:, ft : ft + 1], scale=1.0 / (S_H * S_W),
                    )

                out_sb = ph_f.tile([128, RG, D], F32, tag="out_sb", bufs=1)
                for half in range(2):
                    acc = [
                        ps_acc.tile([128, 512], F32, tag=f"ps_o{rg}",
                                    name=f"ps_o{rg}_{half}")
                        for rg in range(RG)
                    ]
                    for ft in range(NFFT):
                        w_t = ph_f.tile([128, 512], BF16, tag="w_fc2t", bufs=4)
                        nc.sync.dma_start(
                            w_t[:], wfc2_v[:, ft, ds(half * 512, 512)]
                        )
                        for rg in range(RG):
                            nc.tensor.matmul(
                                acc[rg][:], mT[:, ft, ds(rg * 128, 128)], w_t[:],
                                start=(ft == 0), stop=(ft == NFFT - 1),
                            )
                    for rg in range(RG):
                        tmp = ph_f.tile([128, 512], F32, tag="o_tmp")
                        nc.vector.tensor_tensor(
                            tmp[:], acc[rg][:], bfc2[:, ds(half * 512, 512)], ALU.add
                        )
                        nc.vector.tensor_tensor(
                            out_sb[:, rg, ds(half * 512, 512)], tmp[:],
                            x2_sb[:, rg, ds(half * 512, 512)], ALU.add,
                        )
                        nc.sync.dma_start(
                            out_v[:, rg, ds(half * 512, 512)],
                            out_sb[:, rg, ds(half * 512, 512)],
                        )

    nc.compile()
    return nc


def _prep_inputs(inputs):
    """Host-side shard + cast + LN affine folding + fp8 scaling."""
    bf = ml_dtypes.bfloat16
    f8 = mybir.dt.np(FP8)
    x = np.asarray(inputs["x"], np.float32).reshape(T, D)
    ln1s = np.asarray(inputs["ln1_scale"], np.float32)
    ln1b = np.asarray(inputs["ln1_bias"], np.float32)
    ln2s = np.asarray(inputs["ln2_scale"], np.float32)
    ln2b = np.asarray(inputs["ln2_bias"], np.float32)
    w_attn_f = np.asarray(inputs["W_attn"], np.float32)
    w_fc_f = np.asarray(inputs["W_fc"], np.float32)
    b_attn_f = np.asarray(inputs["b_attn"], np.float32) + ln1b @ w_attn_f
    b_fc_f = np.asarray(inputs["b_fc"], np.float32) + ln2b @ w_fc_f

    w_attn_s = ln1s[:, None] * w_attn_f
    w_qk = (w_attn_s[:, : 2 * D] * S_W).astype(f8)
    # v / proj weights rearranged to [p, half, dt, 512] (contiguous
    # DoubleRow planes), fp8 * S_W
    w_v = np.ascontiguousarray(
        (w_attn_s[:, 2 * D :] * S_W).astype(f8)
        .reshape(DT, 128, 2, 512).transpose(1, 2, 0, 3)
    )
    w_proj = np.ascontiguousarray(
        (np.asarray(inputs["W_proj"], np.float32) * S_W).astype(f8)
        .reshape(DT, 128, 2, 512).transpose(1, 2, 0, 3)
    )
    w_fc = (ln2s[:, None] * w_fc_f * S_W).astype(f8)
    w_fc2 = np.asarray(inputs["W_fc2"], np.float32).astype(bf)
    b_qk = np.ascontiguousarray(
        (b_attn_f[: 2 * D] * S_QKV).reshape(16, 128).T
    )
    bv_bc = np.broadcast_to(
        b_attn_f[2 * D :] * S_QKV, (128, D)
    ).copy().astype(np.float32)
    b_fc = np.ascontiguousarray(b_fc_f.reshape(NFFT, 128).T)
    bproj = np.broadcast_to(np.asarray(inputs["b_proj"], np.float32), (128, D)).copy()
    bfc2 = np.broadcast_to(np.asarray(inputs["b_fc2"], np.float32), (128, D)).copy()
    # additive causal mask for a diagonal tile of S^T (rows=keys,
    # cols=queries): invalid where key > query
    ntri = np.where(
        np.triu(np.ones((128, 128), bool)), 0.0, -1e9
    ).astype(np.float32)
    ident = np.eye(128, dtype=np.float32).astype(bf)

    shared = dict(
        w_qk=w_qk, w_v=w_v, w_proj=w_proj, w_fc=w_fc, w_fc2=w_fc2,
        b_qk=b_qk, b_fc=b_fc, bv_bc=bv_bc,
        bproj_bc=bproj, bfc2_bc=bfc2, ntri=ntri, ident=ident,
    )
    return [
        {"x_c": np.ascontiguousarray(x[c * CHUNK : (c + 1) * CHUNK]), **shared}
        for c in range(NCORES)
    ]


def kernel(**inputs) -> np.ndarray:
    if "nc" not in _cached:
        _cached["nc"] = _build_nc()
    nc = _cached["nc"]
    in_maps = _prep_inputs(inputs)
    res = bass_utils.run_bass_kernel_spmd(
        nc, in_maps, core_ids=list(range(NCORES))
    )
    out = np.concatenate(
        [res.results[c]["out_c"] for c in range(NCORES)], axis=0
    )
    return out.reshape(1, T, D).astype(np.float32)
